# revision 3
# baseline (speedup 1.0000x reference)
"""Trainium2 Bass kernel for CalibrationFreeFP8Linear.

Computes: quantize x and w to fp8-e4m3 with EMA-updated dynamic absmax
scales, fp8 matmul (fp32 accumulate), dequantize, cast to bf16.

Sharding: data-parallel over the 16384 (B*S) rows of x across 8 cores;
weight replicated.  TWO small collectives:
  - an early AllGather of each core's 2-chunk w-shard absmax (w is
    replicated, so the 16 w chunks are absmax-scanned cooperatively,
    2 per core; the host rotates the K-chunk order of both x and w by
    2*core so the SPMD program always scans chunks 0..1) -> s_w is
    known ~35us in, and ALL of w quantizes on the ACT engine while the
    x collective is still in flight.
  - the x absmax AllGather right after the x absmax chain finishes.

Host side packs operands per-partition-contiguous ([128, KT, M] with
partition p holding k-rows {j*128+p}) so bulk loads issue as 2 MiB
transfers in FIFO order on the sync ring (the 1 MiB w-shard goes on the
ACT ring concurrently), giving staggered group arrivals that the absmax
chain pipelines under.

absmax on DVE: pairwise tensor_tensor(abs_max) accumulate (~1.23us per
chunk at the 16-bit 2x rate) with a final in-place tensor_tensor_reduce
folding the last chunk AND producing the [P,1] per-partition max in one
op.  No gpsimd partition reduce pre-collective: per-partition vectors
are gathered and reduced post-gather on DVE ([1, 1024] view).

gpsimd's ucode library is warmed by a dummy partition_broadcast at t~2us
(otherwise a ~10us lazy LIBRARY_RELOAD lands in front of the collective
trigger).  Junk matmuls paced by load-group arrivals keep the PE's HAM
clock-gate warm through the prologue so real matmuls run at 2.4 GHz.

Matmul: fp8 DoubleRow m->kk->n with 2 m-tiles of PSUM in flight; the PE
starts on the first quantized pair (~57us) and from then on the xf
quant pipeline (DVE+ACT) is slightly slower than the PE, so the PE
never stalls once started.  Dequant epilogue alternates ACT/DVE; stores
ride the sync ring (last m-tile split across sync+ACT rings).
"""

import numpy as np
import ml_dtypes

import concourse.bass as bass
import concourse.mybir as mybir
import concourse.tile as tile
from concourse import bacc, bass_isa
from concourse.bass import ts
from concourse.bass_utils import run_bass_kernel_spmd

FP8_MAX = 448.0
EMA = 0.9
N_CORES = 8
P = 128

# Full problem shapes (hardcoded; kernel.py must be self-contained).
B, S, K, N = 4, 4096, 2048, 2048
M_PER_CORE = (B * S) // N_CORES  # 2048

# ---- tuning flags
# absmax mode, in fallback order (first one that compiles is used):
#   tt:      DVE tensor_tensor(abs_max) chain + tensor_tensor_reduce final
#   ttr:     DVE tensor_tensor_reduce chain (1x rate, known-good)
#   reduce:  plain tensor_reduce per chunk (known-good baseline)
ABSMAX_MODES = ("tt", "ttr", "reduce")
XG = 4                    # x chunks per bulk transfer (4 -> 2 MiB)
W_SHARD = 2               # w chunks absmax-scanned per core (16/8)
QUANT_RATES = {"dve": 1.2, "act": 2.0}  # us/chunk, for the greedy split
N_WQ_DVE = 8              # w chunks quantized on DVE (rest on ACT)
JUNK_MM = True            # paced junk matmuls to keep the PE HAM-warm


def build_nc(M, Kd, Nd, n_cores=N_CORES, absmax_mode="tt"):
    """Build the SPMD Bass program for one core's [M, Kd] @ [Kd, Nd]^T shard.

    DRAM inputs (per core, chunk-rotated by 2*core host-side):
      xt [P, KT*M] bf16   xt[p, j*M+m] = x[m, k=((j+2c)%KT)*P+p]
      wt [P, KT*Nd] bf16  wt[p, j*Nd+n] = w[n, same k]
      in_s [1] f32, w_s [1] f32.
    Output: out [P, MT*Nd] bf16, out[p, mo*Nd+n] = out_row(mo*P+p, n).
    """
    dt = mybir.dt
    KT = Kd // P           # 16 k-chunks
    MT = M // P            # 16 m-tiles
    N_TILE = min(512, Nd)
    NT = Nd // N_TILE      # 4 n-tiles
    assert Kd % P == 0 and M % P == 0 and Nd % N_TILE == 0
    assert KT % 2 == 0, "DoubleRow needs an even number of k-subtiles"
    assert (KT - W_SHARD) % 2 == 0

    nc = bacc.Bacc(
        "TRN2",
        target_bir_lowering=False,
        debug=False,
        num_devices=n_cores,
    )

    xt = nc.dram_tensor("xt", [P, KT * M], dt.bfloat16, kind="ExternalInput").ap()
    wt = nc.dram_tensor("wt", [P, KT * Nd], dt.bfloat16, kind="ExternalInput").ap()
    in_s = nc.dram_tensor("in_s", [1], dt.float32, kind="ExternalInput").ap()
    w_s = nc.dram_tensor("w_s", [1], dt.float32, kind="ExternalInput").ap()
    out = nc.dram_tensor("out", [P, MT * Nd], dt.bfloat16, kind="ExternalOutput").ap()

    xt_v = xt.rearrange("p (j m) -> p j m", j=KT)
    wt_v = wt.rearrange("p (j n) -> p j n", j=KT)
    out_v = out.rearrange("p (mo n) -> p mo n", mo=MT)

    rg = [list(range(n_cores))]
    MX = mybir.AluOpType.max
    MN = mybir.AluOpType.min
    ABSMX = mybir.AluOpType.abs_max
    AXX = mybir.AxisListType.X
    AXXY = mybir.AxisListType.XY

    # load groups: x in XG-chunk transfers on the sync ring; the w shard
    # rides the ACT ring concurrently; the rest of w follows x on sync.
    x_groups = [list(range(g, min(g + XG, KT))) for g in range(0, KT, XG)]
    w_rest = []
    g = W_SHARD
    while g < KT:
        w_rest.append(list(range(g, min(g + XG, KT))))
        g += XG

    with tile.TileContext(nc) as tc:
        with (
            tc.tile_pool(name="stats", bufs=1) as stats,
            tc.tile_pool(name="dram", bufs=1, space="DRAM") as dram,
            tc.tile_pool(name="xb_pool", bufs=1) as xb_pool,
            tc.tile_pool(name="wb_pool", bufs=1) as wb_pool,
            tc.tile_pool(name="wf_pool", bufs=1) as wf_pool,
            tc.tile_pool(name="xf_pool", bufs=1) as xf_pool,
            tc.tile_pool(name="psum", bufs=max(1, 8 // NT), space="PSUM") as psum,
            tc.tile_pool(name="outp", bufs=3) as outp,
        ):
            # ---- EMA prev scales on the gpsimd SWDGE ring + an early dummy
            # partition_broadcast to force gpsimd's lazy LIBRARY_RELOAD now
            # instead of in front of the collective trigger.
            pv = stats.tile([1, 2], dt.float32)
            nc.gpsimd.dma_start(pv[:, 0:1], in_s.rearrange("(o p) -> p o", p=1))
            nc.gpsimd.dma_start(pv[:, 1:2], w_s.rearrange("(o p) -> p o", p=1))
            warmj = stats.tile([4, 2], dt.float32)
            nc.gpsimd.partition_broadcast(warmj, pv, channels=4)
            p9 = stats.tile([1, 2], dt.float32)
            nc.vector.tensor_scalar_mul(p9, pv, EMA)

            # ---- bulk loads
            xb = xb_pool.tile([P, KT, M], dt.bfloat16)
            wb = wb_pool.tile([P, KT, Nd], dt.bfloat16)
            nc.scalar.dma_start(wb[:, 0:W_SHARD], wt_v[:, 0:W_SHARD])
            for grp in x_groups:
                a, b = grp[0], grp[-1] + 1
                nc.sync.dma_start(xb[:, a:b], xt_v[:, a:b])
            for grp in w_rest:
                a, b = grp[0], grp[-1] + 1
                nc.sync.dma_start(wb[:, a:b], wt_v[:, a:b])

            def src2d(buf, j):
                return buf[:, ts(j, 1)].rearrange("p a b -> p (a b)")

            # ---- absmax.  aw/ax are [P,1] per-partition absmaxes.
            aw = stats.tile([P, 1], dt.float32)
            ax = stats.tile([P, 1], dt.float32)
            acc = stats.tile([P, max(M, Nd)], dt.bfloat16)

            if absmax_mode == "tt":
                # w shard: one ttr op: acc=|max(|w0|,|w1|)|, aw=rowmax(acc)
                nc.vector.tensor_tensor_reduce(
                    out=acc[:, :Nd], in0=src2d(wb, 0), in1=src2d(wb, 1),
                    scale=1.0, scalar=0.0, op0=ABSMX, op1=MX, accum_out=aw,
                )
                # x: pairwise tt chain, final chunk folded via in-place ttr
                nc.vector.tensor_tensor(
                    acc[:, :M], src2d(xb, 0), src2d(xb, 1), ABSMX
                )
                for j in range(2, KT - 1):
                    nc.vector.tensor_tensor(
                        acc[:, :M], acc[:, :M], src2d(xb, j), ABSMX
                    )
                nc.vector.tensor_tensor_reduce(
                    out=acc[:, :M], in0=acc[:, :M], in1=src2d(xb, KT - 1),
                    scale=1.0, scalar=0.0, op0=ABSMX, op1=MX, accum_out=ax,
                )
            elif absmax_mode == "ttr":
                aping = stats.tile([P, 2], dt.float32)

                def chain(buf, js, final):
                    prev = 0.0
                    for idx, j in enumerate(js):
                        nxt = final if idx == len(js) - 1 else (
                            aping[:, idx % 2 : idx % 2 + 1]
                        )
                        F = buf.shape[-1]
                        nc.vector.tensor_tensor_reduce(
                            out=acc[:, :F], in0=src2d(buf, j), in1=src2d(buf, j),
                            scale=1.0, scalar=prev, op0=ABSMX, op1=MX,
                            accum_out=nxt,
                        )
                        prev = nxt

                chain(wb, range(W_SHARD), aw)
                chain(xb, range(KT), ax)
            else:  # "reduce"
                rc_x = stats.tile([P, KT], dt.float32)
                rc_w = stats.tile([P, W_SHARD], dt.float32)
                for j in range(W_SHARD):
                    nc.vector.tensor_reduce(
                        rc_w[:, j : j + 1], src2d(wb, j), axis=AXX, op=MX,
                        apply_absolute_value=True,
                    )
                nc.vector.tensor_reduce(aw, rc_w, axis=AXX, op=MX)
                for j in range(KT):
                    nc.vector.tensor_reduce(
                        rc_x[:, j : j + 1], src2d(xb, j), axis=AXX, op=MX,
                        apply_absolute_value=True,
                    )
                nc.vector.tensor_reduce(ax, rc_x, axis=AXX, op=MX)

            # NOTE on DVE emission order: the w absmax ops are first (the w
            # shard lands before x), then the x chain; the w readback-reduce
            # + w scale chain are emitted mid-x-chain (below) so they run as
            # soon as the w mesh finishes without stalling the x chain.

            # ---- w collective (early): AllGather each core's [P] w-shard
            # absmax; stat write + readback on the ACT ring.
            cc_w_in = dram.tile([P], dt.float32)
            nc.scalar.dma_start(cc_w_in.rearrange("(o p) -> p o", p=P), aw)
            cc_w_out = dram.tile([n_cores * P], dt.float32, addr_space="Shared")
            nc.gpsimd.collective_compute(
                "AllGather", mybir.AluOpType.bypass, replica_groups=rg,
                ins=[cc_w_in.opt()], outs=[cc_w_out.opt()],
            )
            gath = stats.tile([1, n_cores * P], dt.float32)
            nc.scalar.dma_start(
                gath, cc_w_out.rearrange("(o c p) -> o c p", o=1, p=P)
            )

            # hack: the above absmax section emitted the full x chain on the
            # DVE queue; the w scale chain below lands after it but all its
            # inputs are ready by then only if the mesh beat the x chain.
            # That is the common case (w mesh ends ~35us, x chain ~40us).
            swf = stats.tile([1, 1], dt.float32)
            nc.vector.tensor_reduce(
                swf, gath.rearrange("o (c p) -> o c p", p=P), axis=AXXY, op=MX
            )
            nc.vector.tensor_scalar_add(swf, swf, 1e-12)
            nc.vector.reciprocal(swf, swf)
            nc.vector.tensor_scalar_mul(swf, swf, FP8_MAX)
            nc.vector.tensor_scalar(swf, swf, 1e-6, 1e6, MX, MN)
            nc.vector.tensor_scalar_mul(swf, swf, float(1.0 - EMA))
            nc.vector.tensor_add(swf, swf, p9[:, 1:2])
            sb_w = stats.tile([P, 1], dt.float32)
            nc.gpsimd.partition_broadcast(sb_w, swf, channels=P)

            # ---- x collective: stat write + trigger + readback all on the
            # gpsimd SWDGE ring (the ACT ring is busy with w quant by then).
            cc_x_in = dram.tile([P], dt.float32)
            nc.gpsimd.dma_start(cc_x_in.rearrange("(o p) -> p o", p=P), ax)
            cc_x_out = dram.tile([n_cores * P], dt.float32, addr_space="Shared")
            nc.gpsimd.collective_compute(
                "AllGather", mybir.AluOpType.bypass, replica_groups=rg,
                ins=[cc_x_in.opt()], outs=[cc_x_out.opt()],
            )
            gath_x = stats.tile([1, n_cores * P], dt.float32)
            nc.gpsimd.dma_start(
                gath_x, cc_x_out.rearrange("(o c p) -> o c p", o=1, p=P)
            )

            # ---- w quantize: ACT takes the first chunks right after sb_w
            # (~37us); DVE joins after its x absmax chain ends (~42us).
            wf = wf_pool.tile([P, KT, Nd], dt.float8e4)
            xf = xf_pool.tile([P, KT, M], dt.float8e4)
            wq_dve = list(range(KT - N_WQ_DVE, KT))
            for j in range(KT):
                if j in wq_dve:
                    nc.vector.tensor_scalar_mul(
                        wf[:, ts(j, 1)], wb[:, ts(j, 1)], sb_w
                    )
                else:
                    nc.scalar.mul(src2d(wf, j), src2d(wb, j), mul=sb_w)

            # ---- x scale chain (after the x mesh readback) + inv
            sxv = stats.tile([1, 2], dt.float32)
            nc.vector.tensor_reduce(
                sxv[:, 0:1], gath_x.rearrange("o (c p) -> o c p", p=P),
                axis=AXXY, op=MX,
            )
            nc.vector.tensor_scalar_add(sxv[:, 0:1], sxv[:, 0:1], 1e-12)
            nc.vector.reciprocal(sxv[:, 0:1], sxv[:, 0:1])
            nc.vector.tensor_scalar_mul(sxv[:, 0:1], sxv[:, 0:1], FP8_MAX)
            nc.vector.tensor_scalar(sxv[:, 0:1], sxv[:, 0:1], 1e-6, 1e6, MX, MN)
            nc.vector.tensor_scalar_mul(sxv[:, 0:1], sxv[:, 0:1], float(1.0 - EMA))
            nc.vector.tensor_add(sxv[:, 0:1], sxv[:, 0:1], p9[:, 0:1])
            # inv = 1/(s_x*s_w)
            nc.vector.tensor_mul(sxv[:, 1:2], sxv[:, 0:1], swf)
            nc.vector.reciprocal(sxv[:, 1:2], sxv[:, 1:2])
            sb_x = stats.tile([P, 2], dt.float32)
            nc.gpsimd.partition_broadcast(sb_x, sxv, channels=P)
            s_x, inv = sb_x[:, 0:1], sb_x[:, 1:2]

            # ---- x quantize: greedy DVE/ACT split in consumption order
            clocks = {"dve": 0.0, "act": 0.0}
            for j in range(KT):
                eng = min(clocks, key=lambda e: clocks[e] + QUANT_RATES[e])
                clocks[eng] += QUANT_RATES[eng]
                if eng == "dve":
                    nc.vector.tensor_scalar_mul(
                        xf[:, ts(j, 1)], xb[:, ts(j, 1)], s_x
                    )
                else:
                    nc.scalar.mul(src2d(xf, j), src2d(xb, j), mul=s_x)

            # ---- fp8 DoubleRow matmul + dequant epilogue
            for m in range(MT):
                pts = [
                    psum.tile([P, N_TILE], dt.float32, name=f"pt{n}") for n in range(NT)
                ]
                if m == 0 and JUNK_MM:
                    # paced junk matmuls (one per load-group / late-wf-chunk
                    # arrival) keep the PE HAM-warm through the prologue.
                    pace = [(wb, 0), (xb, 0)]
                    pace += [(xb, grp[0]) for grp in x_groups[1:]]
                    pace += [(wb, grp[0]) for grp in w_rest]
                    pace += [(wf, j) for j in wq_dve[-4:]]
                    for buf, j in pace:
                        nc.tensor.matmul(
                            pts[0], buf[:, j, 0:P], buf[:, j, 0:N_TILE],
                            start=True, stop=True, skip_group_check=True,
                        )
                for kk in range(KT // 2):
                    for n in range(NT):
                        nc.tensor.matmul(
                            pts[n],
                            xf[:, 2 * kk : 2 * kk + 2, ts(m, P)],
                            wf[:, 2 * kk : 2 * kk + 2, ts(n, N_TILE)],
                            start=(kk == 0),
                            stop=(kk == KT // 2 - 1),
                            perf_mode=mybir.MatmulPerfMode.DoubleRow,
                        )
                for n in range(NT):
                    out_mn = outp.tile([P, N_TILE], dt.bfloat16, name="out_mn")
                    if n % 2 == 0:
                        nc.scalar.mul(out_mn, pts[n], mul=inv)
                    else:
                        nc.vector.tensor_scalar_mul(out_mn, pts[n], inv)
                    if m == MT - 1 and n % 2 == 1:
                        nc.scalar.dma_start(out_v[:, m, ts(n, N_TILE)], out_mn)
                    else:
                        nc.sync.dma_start(out_v[:, m, ts(n, N_TILE)], out_mn)

    nc.compile()
    return nc


_NC_CACHE = {}
_WORKING_MODE = [0]


def _get_nc(M, Kd, Nd, n_cores=N_CORES, mode_idx=0):
    key = (M, Kd, Nd, n_cores, mode_idx)
    if key not in _NC_CACHE:
        _NC_CACHE[key] = build_nc(
            M, Kd, Nd, n_cores, absmax_mode=ABSMAX_MODES[mode_idx]
        )
    return _NC_CACHE[key]


def _pack_rotated(a_km, KT, rot):
    """[K, F] -> [P, KT*F] with chunk j holding original chunk (j+rot)%KT,
    per-partition contiguous."""
    Kd, F = a_km.shape
    a = a_km.reshape(KT, P, F)
    if rot:
        a = np.roll(a, -rot, axis=0)
    return np.ascontiguousarray(a.transpose(1, 0, 2)).reshape(P, KT * F)


def run_sharded(x2d, weight, input_scale, weight_scale, n_cores=N_CORES, trace=False):
    """x2d: [rows, K] bf16, weight: [N, K] bf16. Returns ([rows, N] bf16, result)."""
    rows, k = x2d.shape
    n = weight.shape[0]
    m_per = rows // n_cores
    KT = k // P
    MT = m_per // P
    wT = np.ascontiguousarray(weight.T)  # [K, N]
    in_s = np.asarray(input_scale, dtype=np.float32).reshape(1)
    w_s = np.asarray(weight_scale, dtype=np.float32).reshape(1)
    in_maps = []
    for i in range(n_cores):
        rot = (2 * i) % KT
        xt_i = _pack_rotated(
            np.ascontiguousarray(x2d[i * m_per : (i + 1) * m_per].T), KT, rot
        )
        wt_i = _pack_rotated(wT, KT, rot)
        in_maps.append({"xt": xt_i, "wt": wt_i, "in_s": in_s, "w_s": w_s})

    last_err = None
    start_idx = _WORKING_MODE[0]
    for mode_idx in range(start_idx, len(ABSMAX_MODES)):
        try:
            nc = _get_nc(m_per, k, n, n_cores, mode_idx)
            res = run_bass_kernel_spmd(
                nc, in_maps, core_ids=list(range(n_cores)), trace=trace
            )
            _WORKING_MODE[0] = mode_idx
            break
        except Exception as e:  # fall back to the next absmax mode
            last_err = e
            _NC_CACHE.pop((m_per, k, n, n_cores, mode_idx), None)
            if mode_idx == len(ABSMAX_MODES) - 1:
                raise
    outs = []
    for i in range(n_cores):
        o = res.results[i]["out"].reshape(P, MT, n)
        outs.append(o.transpose(1, 0, 2).reshape(m_per, n))
    out = np.concatenate(outs, axis=0)
    return out, res


def kernel(x, weight, input_scale, weight_scale):
    x = np.asarray(x)
    weight = np.asarray(weight)
    b, s, k = x.shape
    x2d = np.ascontiguousarray(x.reshape(b * s, k))
    out, _ = run_sharded(x2d, weight, input_scale, weight_scale)
    return out.reshape(b, s, weight.shape[0]).astype(ml_dtypes.bfloat16)


# revision 6
# speedup vs baseline: 1.0464x; 1.0464x over previous
"""Trainium2 Bass kernel for CalibrationFreeFP8Linear.

Computes: quantize x and w to fp8-e4m3 with EMA-updated dynamic absmax
scales, fp8 matmul (fp32 accumulate), dequantize, cast to bf16.

Sharding: data-parallel over the 16384 (B*S) rows of x across 8 cores;
weight replicated.  One combined AllGather carries each core's
per-partition x absmax plus the absmax of a 2-chunk shard of w (w is
replicated, so the 16 w chunks are absmax-scanned cooperatively; the
host rotates the K-chunk order of both x and w by 2*core so the SPMD
program always scans w chunks 0..1 -- identical program per core).

KEY TRICK -- dummy warm-up collective: the FIRST collective in a NEFF
pays a ~70-100us ncfw/TOPSP warm-up (measured: the first mesh ends at
~104us regardless of when its input was ready; a second collective
completes in ~13us).  A zero-payload AllGather triggered at t~4us
absorbs that cost so the real one runs at intrinsic latency.

Host side packs operands per-partition-contiguous ([128, KT, M] with
partition p holding k-rows {j*128+p}) so bulk loads issue as 2 MiB
transfers in FIFO order on the sync ring (the 1 MiB w-shard rides the
ACT ring concurrently), giving staggered group arrivals.

absmax: abs_max-flavored DVE ops run at the 1x rate (~2.2us/chunk) but
plain MAX tensor_tensor runs at 2x (~1.23us).  So most x chunks go
ACT-abs (1.9us, half-chunk ping-pong) + DVE tt-MAX accumulate (1.23us),
a few go direct DVE tensor_reduce, balancing both engines so the absmax
finishes ~1 chunk after the last x load group lands.

Then: stat write + readback on the ACT ring, [1,1024] DVE reduces,
EMA scales + inv on tiny tiles, one partition_broadcast; quantize all
32 chunks in matmul consumption order split DVE/ACT; fp8 DoubleRow
matmuls m->kk->n (2 m-tiles of PSUM in flight) start on the first
quantized pair and never stall; dequant epilogue alternates ACT/DVE;
stores ride the sync ring (last m-tile split across sync+ACT rings).
"""

import numpy as np
import ml_dtypes

import concourse.bass as bass
import concourse.mybir as mybir
import concourse.tile as tile
from concourse import bacc, bass_isa
from concourse.bass import ts
from concourse.bass_utils import run_bass_kernel_spmd

FP8_MAX = 448.0
EMA = 0.9
N_CORES = 8
P = 128

B, S, K, N = 4, 4096, 2048, 2048
M_PER_CORE = (B * S) // N_CORES  # 2048

# absmax mode fallback order (first one that compiles+runs is used):
#   mixed:  ACT half-chunk abs + DVE tt-MAX for most chunks, direct DVE
#           tensor_reduce for the rest (fastest)
#   ttr:    DVE tensor_tensor_reduce chain (1x rate, known-good)
#   reduce: plain tensor_reduce per chunk (known-good baseline)
ABSMAX_MODES = ("mixed", "reduce")
XG = 4                    # x chunks per bulk transfer (4 -> 2 MiB)
W_SHARD = 2               # w chunks absmax-scanned per core (16/8)
QUANT_RATES = {"dve": 1.2, "act": 2.0}  # us/chunk for the greedy split
DUMMY_CC = True           # warm-up collective at t~4us
# direct-DVE x chunks for "mixed" (first chunks of early groups so the
# DVE has work while the ACT abs pipeline fills; rest go ACT-path)
DIRECT_X = (0, 4, 8)


def build_nc(M, Kd, Nd, n_cores=N_CORES, absmax_mode="mixed"):
    dt = mybir.dt
    KT = Kd // P
    MT = M // P
    N_TILE = min(512, Nd)
    NT = Nd // N_TILE
    assert Kd % P == 0 and M % P == 0 and Nd % N_TILE == 0
    assert KT % 2 == 0

    nc = bacc.Bacc(
        "TRN2",
        target_bir_lowering=False,
        debug=False,
        num_devices=n_cores,
    )

    xt = nc.dram_tensor("xt", [P, KT * M], dt.bfloat16, kind="ExternalInput").ap()
    wt = nc.dram_tensor("wt", [P, KT * Nd], dt.bfloat16, kind="ExternalInput").ap()
    in_s = nc.dram_tensor("in_s", [1], dt.float32, kind="ExternalInput").ap()
    w_s = nc.dram_tensor("w_s", [1], dt.float32, kind="ExternalInput").ap()
    out = nc.dram_tensor("out", [P, MT * Nd], dt.bfloat16, kind="ExternalOutput").ap()

    xt_v = xt.rearrange("p (j m) -> p j m", j=KT)
    wt_v = wt.rearrange("p (j n) -> p j n", j=KT)
    out_v = out.rearrange("p (mo n) -> p mo n", mo=MT)

    rg = [list(range(n_cores))]
    MX = mybir.AluOpType.max
    MN = mybir.AluOpType.min
    ABSMX = mybir.AluOpType.abs_max
    AXX = mybir.AxisListType.X

    x_groups = [list(range(g, min(g + XG, KT))) for g in range(0, KT, XG)]
    w_rest = []
    g = W_SHARD
    while g < KT:
        w_rest.append(list(range(g, min(g + XG, KT))))
        g += XG

    with tile.TileContext(nc) as tc:
        with (
            tc.tile_pool(name="stats", bufs=1) as stats,
            tc.tile_pool(name="dram", bufs=1, space="DRAM") as dram,
            tc.tile_pool(name="xb_pool", bufs=1) as xb_pool,
            tc.tile_pool(name="wb_pool", bufs=1) as wb_pool,
            tc.tile_pool(name="wf_pool", bufs=1) as wf_pool,
            tc.tile_pool(name="xf_pool", bufs=1) as xf_pool,
            tc.tile_pool(name="psum", bufs=max(1, 8 // NT), space="PSUM") as psum,
            tc.tile_pool(name="outp", bufs=3) as outp,
        ):
            # ---- dummy warm-up collective (see module docstring)
            if DUMMY_CC and n_cores > 1:
                z2 = stats.tile([1, 2], dt.float32)
                nc.vector.memset(z2, 0.0)
                cc_d_in = dram.tile([2], dt.float32)
                nc.scalar.dma_start(cc_d_in.rearrange("(o t) -> o t", o=1), z2)
                cc_d_out = dram.tile([n_cores * 2], dt.float32, addr_space="Shared")
                nc.gpsimd.collective_compute(
                    "AllGather", mybir.AluOpType.bypass, replica_groups=rg,
                    ins=[cc_d_in.opt()], outs=[cc_d_out.opt()],
                )

            # EMA prev scales on the gpsimd SWDGE ring; p9 = 0.9*prev
            pv = stats.tile([1, 2], dt.float32)
            nc.gpsimd.dma_start(pv[:, 0:1], in_s.rearrange("(o p) -> p o", p=1))
            nc.gpsimd.dma_start(pv[:, 1:2], w_s.rearrange("(o p) -> p o", p=1))
            p9 = stats.tile([1, 2], dt.float32)
            nc.vector.tensor_scalar_mul(p9, pv, EMA)

            # ---- bulk loads: w shard on the ACT ring, x + w-rest on sync
            xb = xb_pool.tile([P, KT, M], dt.bfloat16)
            wb = wb_pool.tile([P, KT, Nd], dt.bfloat16)
            nc.scalar.dma_start(wb[:, 0:W_SHARD], wt_v[:, 0:W_SHARD])
            for grp in x_groups:
                a, b = grp[0], grp[-1] + 1
                nc.sync.dma_start(xb[:, a:b], xt_v[:, a:b])
            for grp in w_rest:
                a, b = grp[0], grp[-1] + 1
                nc.sync.dma_start(wb[:, a:b], wt_v[:, a:b])

            def src2d(buf, j):
                return buf[:, ts(j, 1)].rearrange("p a b -> p (a b)")

            # ---- absmax -> aw, ax [P,1].  (abs_max AluOp does not exist in
            # the ISA codegen; tensor_reduce(apply_absolute_value) is 1x rate
            # ~2.2us/chunk, ACT Abs activation ~0.95us/half + DVE plain-MAX
            # tensor_tensor ~0.65us/half is the fast path.)
            aw = stats.tile([P, 1], dt.float32)
            ax = stats.tile([P, 1], dt.float32)

            # w shard: two direct DVE reduces + tiny combine
            rc_w = stats.tile([P, W_SHARD], dt.float32)
            for j in range(W_SHARD):
                nc.vector.tensor_reduce(
                    rc_w[:, j : j + 1], src2d(wb, j), axis=AXX, op=MX,
                    apply_absolute_value=True,
                )
            nc.vector.tensor_reduce(aw, rc_w, axis=AXX, op=MX)

            if absmax_mode == "mixed":
                H = M // 2
                acc = stats.tile([P, H], dt.bfloat16)
                ab = stats.tile([P, 2, H], dt.bfloat16)  # half-chunk ping-pong
                rc = stats.tile([P, len(DIRECT_X)], dt.float32)
                di = 0
                first_tt = True
                hidx = 0
                for j in range(KT):
                    if j in DIRECT_X:
                        nc.vector.tensor_reduce(
                            rc[:, di : di + 1], src2d(xb, j), axis=AXX, op=MX,
                            apply_absolute_value=True,
                        )
                        di += 1
                    else:
                        ch = src2d(xb, j)
                        for h in range(2):
                            abh = ab[:, hidx % 2]
                            hidx += 1
                            nc.scalar.activation(
                                abh, ch[:, h * H : (h + 1) * H],
                                mybir.ActivationFunctionType.Abs,
                            )
                            if first_tt:
                                nc.vector.tensor_tensor(acc, abh, abh, MX)
                                first_tt = False
                            else:
                                nc.vector.tensor_tensor(acc, acc, abh, MX)
                t1 = stats.tile([P, 1], dt.float32)
                nc.vector.tensor_reduce(t1, acc, axis=AXX, op=MX)
                t2 = stats.tile([P, 1], dt.float32)
                nc.vector.tensor_reduce(t2, rc, axis=AXX, op=MX)
                nc.vector.tensor_tensor(ax, t1, t2, MX)
            else:  # "reduce"
                rc_x = stats.tile([P, KT], dt.float32)
                for j in range(KT):
                    nc.vector.tensor_reduce(
                        rc_x[:, j : j + 1], src2d(xb, j), axis=AXX, op=MX,
                        apply_absolute_value=True,
                    )
                nc.vector.tensor_reduce(ax, rc_x, axis=AXX, op=MX)

            # ---- combined collective: cc_in = [ax[128] | aw[128]]
            cc_in = dram.tile([2 * P], dt.float32)
            cc_in_v = cc_in.rearrange("(t p) -> p t", p=P)
            nc.scalar.dma_start(cc_in_v[:, 0:1], ax)
            nc.scalar.dma_start(cc_in_v[:, 1:2], aw)
            cc_out = dram.tile([n_cores * 2 * P], dt.float32, addr_space="Shared")
            nc.gpsimd.collective_compute(
                "AllGather", mybir.AluOpType.bypass, replica_groups=rg,
                ins=[cc_in.opt()], outs=[cc_out.opt()],
            )
            # readback both halves through one [1,1024] tile (SBUF-tight):
            # x part -> reduce -> w part -> reduce
            gath = stats.tile([1, n_cores * P], dt.float32)
            cc_rv = cc_out.rearrange("(o c t p) -> o t c p", o=1, t=2, p=P)
            sxw = stats.tile([1, 2], dt.float32)
            nc.scalar.dma_start(gath, cc_rv[:, 0])
            nc.vector.tensor_reduce(sxw[:, 0:1], gath, axis=AXX, op=MX)
            nc.scalar.dma_start(gath, cc_rv[:, 1])
            nc.vector.tensor_reduce(sxw[:, 1:2], gath, axis=AXX, op=MX)

            # ---- EMA scales on [1,2]: s = 0.9*prev + 0.1*clip(448/(a+eps))
            nc.vector.tensor_scalar_add(sxw, sxw, 1e-12)
            nc.vector.reciprocal(sxw, sxw)
            nc.vector.tensor_scalar_mul(sxw, sxw, FP8_MAX)
            nc.vector.tensor_scalar(sxw, sxw, 1e-6, 1e6, MX, MN)
            nc.vector.tensor_scalar_mul(sxw, sxw, float(1.0 - EMA))
            sf = stats.tile([1, 3], dt.float32)
            nc.vector.tensor_add(sf[:, 0:2], sxw, p9)
            nc.vector.tensor_mul(sf[:, 2:3], sf[:, 0:1], sf[:, 1:2])
            nc.vector.reciprocal(sf[:, 2:3], sf[:, 2:3])
            sb = stats.tile([P, 3], dt.float32)
            nc.gpsimd.partition_broadcast(sb, sf, channels=P)
            s_x, s_w, inv = sb[:, 0:1], sb[:, 1:2], sb[:, 2:3]

            # ---- quantize in matmul consumption order, greedy DVE/ACT
            xf = xf_pool.tile([P, KT, M], dt.float8e4)
            wf = wf_pool.tile([P, KT, Nd], dt.float8e4)
            seq = []
            for k in range(KT // 2):
                seq += [("w", 2 * k), ("w", 2 * k + 1),
                        ("x", 2 * k), ("x", 2 * k + 1)]
            clocks = {"dve": 0.0, "act": 0.0}
            for t, j in seq:
                eng = min(clocks, key=lambda e: clocks[e] + QUANT_RATES[e])
                clocks[eng] += QUANT_RATES[eng]
                if t == "x":
                    src, dst, s = xb[:, ts(j, 1)], xf[:, ts(j, 1)], s_x
                else:
                    src, dst, s = wb[:, ts(j, 1)], wf[:, ts(j, 1)], s_w
                if eng == "dve":
                    nc.vector.tensor_scalar_mul(dst, src, s)
                else:
                    nc.scalar.mul(dst.rearrange("p a b -> p (a b)"),
                                  src.rearrange("p a b -> p (a b)"), mul=s)

            # ---- fp8 DoubleRow matmul + dequant epilogue
            for m in range(MT):
                pts = [
                    psum.tile([P, N_TILE], dt.float32, name=f"pt{n}")
                    for n in range(NT)
                ]
                for kk in range(KT // 2):
                    for n in range(NT):
                        nc.tensor.matmul(
                            pts[n],
                            xf[:, 2 * kk : 2 * kk + 2, ts(m, P)],
                            wf[:, 2 * kk : 2 * kk + 2, ts(n, N_TILE)],
                            start=(kk == 0),
                            stop=(kk == KT // 2 - 1),
                            perf_mode=mybir.MatmulPerfMode.DoubleRow,
                        )
                for n in range(NT):
                    out_mn = outp.tile([P, N_TILE], dt.bfloat16, name="out_mn")
                    if n % 2 == 0:
                        nc.scalar.mul(out_mn, pts[n], mul=inv)
                    else:
                        nc.vector.tensor_scalar_mul(out_mn, pts[n], inv)
                    if m == MT - 1 and n % 2 == 1:
                        nc.scalar.dma_start(out_v[:, m, ts(n, N_TILE)], out_mn)
                    else:
                        nc.sync.dma_start(out_v[:, m, ts(n, N_TILE)], out_mn)

    nc.compile()
    return nc


_NC_CACHE = {}
_WORKING_MODE = [0]


def _get_nc(M, Kd, Nd, n_cores=N_CORES, mode_idx=0):
    key = (M, Kd, Nd, n_cores, mode_idx)
    if key not in _NC_CACHE:
        _NC_CACHE[key] = build_nc(
            M, Kd, Nd, n_cores, absmax_mode=ABSMAX_MODES[mode_idx]
        )
    return _NC_CACHE[key]


def _pack_rotated(a_km, KT, rot):
    Kd, F = a_km.shape
    a = a_km.reshape(KT, P, F)
    if rot:
        a = np.roll(a, -rot, axis=0)
    return np.ascontiguousarray(a.transpose(1, 0, 2)).reshape(P, KT * F)


def run_sharded(x2d, weight, input_scale, weight_scale, n_cores=N_CORES, trace=False):
    rows, k = x2d.shape
    n = weight.shape[0]
    m_per = rows // n_cores
    KT = k // P
    MT = m_per // P
    wT = np.ascontiguousarray(weight.T)
    in_s = np.asarray(input_scale, dtype=np.float32).reshape(1)
    w_s = np.asarray(weight_scale, dtype=np.float32).reshape(1)
    in_maps = []
    for i in range(n_cores):
        rot = (2 * i) % KT
        xt_i = _pack_rotated(
            np.ascontiguousarray(x2d[i * m_per : (i + 1) * m_per].T), KT, rot
        )
        wt_i = _pack_rotated(wT, KT, rot)
        in_maps.append({"xt": xt_i, "wt": wt_i, "in_s": in_s, "w_s": w_s})

    last_err = None
    start_idx = _WORKING_MODE[0]
    for mode_idx in range(start_idx, len(ABSMAX_MODES)):
        try:
            nc = _get_nc(m_per, k, n, n_cores, mode_idx)
            res = run_bass_kernel_spmd(
                nc, in_maps, core_ids=list(range(n_cores)), trace=trace
            )
            _WORKING_MODE[0] = mode_idx
            break
        except Exception as e:
            last_err = e
            _NC_CACHE.pop((m_per, k, n, n_cores, mode_idx), None)
            if mode_idx == len(ABSMAX_MODES) - 1:
                raise
    outs = []
    for i in range(n_cores):
        o = res.results[i]["out"].reshape(P, MT, n)
        outs.append(o.transpose(1, 0, 2).reshape(m_per, n))
    out = np.concatenate(outs, axis=0)
    return out, res


def kernel(x, weight, input_scale, weight_scale):
    x = np.asarray(x)
    weight = np.asarray(weight)
    b, s, k = x.shape
    x2d = np.ascontiguousarray(x.reshape(b * s, k))
    out, _ = run_sharded(x2d, weight, input_scale, weight_scale)
    return out.reshape(b, s, weight.shape[0]).astype(ml_dtypes.bfloat16)
